# revision 15
# baseline (speedup 1.0000x reference)
"""Trainium2 Bass kernel for softmax-free attention:
    q = x @ Wq^T; k = x @ Wk^T; v = x @ Wv^T
    s = (q @ k^T) / sqrt(d); out = s @ v
  x: [4, 4096, 1024], W*: [1024, 1024], out: [4, 4096, 1024] (fp32)

No softmax => the whole map is linear and can be re-associated:
    out[b] = x[b] @ A[b],  A[b] = PT^T @ (G[b] @ Wv^T),
    G[b] = x[b]^T x[b],    PT = Wk^T (Wq/sqrt(d))   (host-folded weights)
This cuts device MACs ~4x vs the naive chain (projections + LxL scores).

Sharding: 8 cores; core c handles batch b=c//2, sequence-half h=c%2.  Both
pair members stream the FULL x[b] row-split: each computes the partial gram
G_part = x_half^T x_half over its OWN 2048 rows, spills the 4MB partial to a
cross-core-visible Shared-DRAM slot, and adds the peer's partial (read at
local HBM bandwidth) to form the full G.  Ordering across the pair is a tiny
token AllReduce per spill pass (the token is DMA-sampled from the shared
buffer, so it carries a RAW dep on the pass's spill writes); only the peer
reads wait on it.  After G, the chain V1 = G @ WvT (uses G's symmetry to
keep the contraction on partitions), A = PT^T @ V1 is duplicated across the
pair (cheap: 2 x 27us), and out_half = x_half @ A covers the core's own 2048
output rows.

Layout: the PE contracts over the partition dim; every matmul is arranged so
its output lands partition-major for the next stage:
  G[d,g]   = sum_l  x[l,d] x[l,g]      (lhsT=x chunk col-slice, rhs=x chunk)
  V1[d,d'] = sum_g  G[g,d] WvT[g,d']   (lhsT=G row-chunk == col-chunk, sym!)
  A[e,d']  = sum_d  PT[d,e] V1[d,d']   (lhsT=PT chunk, rhs=V1 chunk)
  out[l,d']= sum_e  xT[e,l] A[e,d']    (lhsT=xT_half chunk, rhs=A chunk)
All matmul inputs are float32r (full PE rate at free-dim>=256, ~1e-4 rel
err); PSUM accumulates fp32.  G's 2048-row partial runs as two passes of 8
resident PSUM banks (4 d-tiles x 2 g-halves x 16 l-chunks each); each pass
spills immediately so the pair exchange overlaps the second pass.
"""

import sys
import types
from contextlib import ExitStack

import numpy as np

import concourse.bass as bass
import concourse.tile as tile
from concourse import bacc, mybir
from concourse.bass_utils import run_bass_kernel_spmd
from concourse.mybir import EngineType
from concourse.tile import add_dep_helper
from concourse.vector_clock import ScopedClock

# ---------------------------------------------------------------------------
# Environment shims
# ---------------------------------------------------------------------------


def _install_tile_drain_patch():
    """This toolchain's walrus caps sync waits at 1 per instruction, but
    TileContext's tail drain can carry several. Split the overflow onto
    preceding nops (same semantics: the issuing engine observes every sem
    before draining)."""
    if getattr(tile.TileContext, "_drain_patch_installed", False):
        return

    def _patched_drain_and_barrier(self, tick_clock, wait_clock):
        nc = self.nc
        collector = nc.sync.nop(hint="drain_wait_collector", nofuse=True)
        wait_clock.add_sem_waits(
            collector.ins, ScopedClock({None: tick_clock.global_clock})
        )
        waits = list(collector.ins.sync_info.on_wait or [])
        if len(waits) > 1:
            collector.ins.sync_info.on_wait = [waits[0]]
            for w in waits[1:]:
                nop = nc.sync.nop(hint="drain_wait_extra", nofuse=True)
                nop.ins.sync_info = mybir.SyncInfo(on_wait=[w], on_update=[])
        nc.sync.drain()

        nc.all_engine_barrier()
        assert self.sems is not None
        popped = nc._tile_sem_poison_stack.pop()
        assert popped is self._sem_poison
        nc.clear_and_free_semaphores(list(self.sems.allocated().values()))
        nc.all_engine_barrier()

    tile.TileContext._drain_and_barrier = _patched_drain_and_barrier
    tile.TileContext._drain_patch_installed = True


def _install_ntff_shim():
    """The image's antenv lacks axon_hooks, which silently degrades
    trace=True. Recreate the get/set pair and register the ctypes NTFF hook
    from trn_agent_boot (no-op if unavailable)."""
    if "antenv.axon_hooks" in sys.modules:
        return
    state = {"hook": None}

    def set_axon_ntff_profile_hook(h):
        state["hook"] = h

    def get_axon_ntff_profile_hook():
        return state["hook"]

    mod = types.ModuleType("antenv.axon_hooks")
    mod.set_axon_ntff_profile_hook = set_axon_ntff_profile_hook
    mod.get_axon_ntff_profile_hook = get_axon_ntff_profile_hook
    sys.modules["antenv.axon_hooks"] = mod
    try:
        import antenv

        antenv.axon_hooks = mod
        from trn_agent_boot.trn_boot import _ntff_profile_via_ctypes

        set_axon_ntff_profile_hook(
            _ntff_profile_via_ctypes("/opt/axon/libaxon_pjrt.so")
        )
    except Exception:
        pass


_install_tile_drain_patch()
_install_ntff_shim()

# ---------------------------------------------------------------------------
# Problem constants (hardcoded per the harness contract)
# ---------------------------------------------------------------------------

B, L, D = 4, 4096, 1024
N_CORES = 8
P = 128
LH = L // 2  # rows per core
DC = D // P  # 8 chunks of 128 over d/e/g
F32 = mybir.dt.float32
F32R = mybir.dt.float32r
FREE = 512  # rhs free dim per matmul (one PSUM bank)

PAIRS = [[2 * i, 2 * i + 1] for i in range(N_CORES // 2)]
# G partial in 3 passes (PSUM-bank groups of d-tiles); the tail passes are
# small so the last pair exchange has little data and launches late-but-tiny
GDTS = [[0, 1, 2, 3], [4, 5], [6, 7]]
GPASS = len(GDTS)
LCH = LH // P      # 16 l-chunks of own half


def build_nc():
    nc = bacc.Bacc("TRN2", target_bir_lowering=False, debug=False,
                   num_devices=N_CORES)
    xb = nc.dram_tensor("xb", [LH, D], F32, kind="ExternalInput").ap()
    xTh = nc.dram_tensor("xTh", [D, LH], F32, kind="ExternalInput").ap()
    wvT = nc.dram_tensor("wvT", [D, D], F32, kind="ExternalInput").ap()
    ptm = nc.dram_tensor("ptm", [D, D], F32, kind="ExternalInput").ap()
    out = nc.dram_tensor("out", [LH, D], F32, kind="ExternalOutput").ap()
    slots = nc.dram_tensor("slots", [1, 2], mybir.dt.uint32,
                           kind="ExternalInput").ap()
    Gsh = nc.dram_tensor("Gsh", [2, D, D], F32R, addr_space="Shared").ap()
    toks = [nc.dram_tensor(f"tok{p}", [1, 1], F32).ap()
            for p in range(GPASS + 1)]
    toks2 = [nc.dram_tensor(f"tok2{p}", [1, 1], F32).ap()
             for p in range(GPASS + 1)]
    wu_sink = nc.dram_tensor("wu_sink", [P, FREE], F32).ap()

    def chunked(ap):  # [K*, N] dram -> [P, K*/P, N] partition-major
        return ap.rearrange("(c p) n -> p c n", p=P)

    with tile.TileContext(nc) as tc, ExitStack() as octx:
        psum = octx.enter_context(tc.tile_pool(name="psum", bufs=8,
                                               space="PSUM"))
        gpool = octx.enter_context(tc.tile_pool(name="gpool", bufs=1))
        wpool = octx.enter_context(tc.tile_pool(name="wpool", bufs=1))
        mpool = octx.enter_context(tc.tile_pool(name="mpool", bufs=2))

        gsb = gpool.tile([P, DC, D], F32R, tag="gsb")  # full G after add
        wv = wpool.tile([P, DC, D], F32R, tag="wv")
        pt = wpool.tile([P, DC, D], F32R, tag="pt")
        # dummy collective: absorbs the ~11us CC-mesh cold start so the real
        # pair barriers later begin within ~1us of their trigger
        nc.gpsimd.collective_compute(
            "AllReduce", mybir.AluOpType.add, replica_groups=PAIRS,
            ins=[toks[GPASS]], outs=[toks2[GPASS]])
        nc.gpsimd.dma_start(wv[:], chunked(wvT).bitcast(F32R))
        nc.gpsimd.dma_start(pt[:], chunked(ptm).bitcast(F32R))

        # rank-in-pair slot selectors for the shared spill buffer
        st_sl = mpool.tile([1, 2], mybir.dt.uint32, tag="sl", bufs=1)
        nc.sync.dma_start(st_sl[:], slots[:])
        regs_o = nc.alloc_registers(
            engines=[EngineType.SP, EngineType.Activation])
        nc.regs_load(regs_o, st_sl[0:1, 0:1])
        svo = nc.snap(regs_o, donate=True)
        regs_p = nc.alloc_registers(
            engines=[EngineType.SP, EngineType.Activation])
        nc.regs_load(regs_p, st_sl[0:1, 1:2])
        svp = nc.snap(regs_p, donate=True)

        # HAM warmup: junk matmuls while the first DMAs load, so the PE
        # clock gate is already at 8/8 when real work arrives
        barriers = [None] * GPASS
        with ExitStack() as gctx:
            wupool = gctx.enter_context(tc.tile_pool(name="wupool", bufs=1))
            xpool = gctx.enter_context(tc.tile_pool(name="xpool", bufs=1))

            wut = wupool.tile([P, FREE], F32R, tag="wut")
            nc.vector.memset(wut[:].bitcast(F32), 0.0)
            wuo = wupool.tile([P, FREE], F32, tag="wuo")
            for g in range(20):
                wp = psum.tile([P, FREE], F32, tag="ps", name=f"wu_{g}")
                for r in range(2):
                    nc.tensor.matmul(wp[:], wut[:, 0:P], wut[:],
                                     start=(r == 0), stop=(r == 1))
                if g == 19:
                    nc.vector.tensor_copy(wuo[:], wp[:])
            nc.sync.dma_start(wu_sink[:], wuo[:])

            # ------------- Phase G: partial gram over own 2048 rows --------
            xsb = xpool.tile([P, LCH, D], F32R, tag="xsb")
            for c in range(LCH):  # 512KB chunks on 2 rings so lc=0 is early
                eng = nc.scalar if c % 2 == 0 else nc.sync
                eng.dma_start(xsb[:, c:c + 1],
                              chunked(xb).bitcast(F32R)[:, c:c + 1])

            for gp, dts in enumerate(GDTS):
                gps = {}
                for dt in dts:
                    for gh in range(2):
                        gps[dt, gh] = psum.tile([P, FREE], F32, tag="ps",
                                                name=f"g_{dt}_{gh}")
                for lc in range(LCH):
                    for dt in dts:
                        for gh in range(2):
                            nc.tensor.matmul(
                                gps[dt, gh][:],
                                xsb[:, lc, dt * P:(dt + 1) * P],
                                xsb[:, lc, gh * FREE:(gh + 1) * FREE],
                                start=(lc == 0),
                                stop=(lc == LCH - 1))
                for dt in dts:
                    for gh in range(2):
                        nc.vector.tensor_copy(
                            gsb[:, dt, gh * FREE:(gh + 1) * FREE],
                            gps[dt, gh][:])
                # one bulk spill per pass (cheap descriptor gen), then a pair
                # barrier: the token samples the spilled range so its DMA
                # carries a RAW dep on the spill; the AllReduce completes
                # only when BOTH pair members' pass-gp spills are durable
                d0, d1 = dts[0], dts[-1] + 1
                nc.sync.dma_start(
                    Gsh[bass.ds(svo, 1), d0 * P:d1 * P, :].rearrange(
                        "s (c p) n -> p (s c) n", p=P),
                    gsb[:, d0:d1])
                tkt = mpool.tile([1, 1], F32, tag=f"tkt{gp}", bufs=1)
                nc.sync.dma_start(
                    tkt[0:1, 0:1],
                    Gsh[bass.ds(svo, 1), d0 * P:d0 * P + 1, 0:1].rearrange(
                        "s c n -> c s n").bitcast(F32))
                nc.sync.dma_start(toks[gp][:], tkt[0:1, 0:1])
                barriers[gp] = nc.gpsimd.collective_compute(
                    "AllReduce", mybir.AluOpType.add, replica_groups=PAIRS,
                    ins=[toks[gp]], outs=[toks2[gp]])

            # read peer partial chunk-by-chunk and add into gsb; V1 below
            # consumes chunk gc as soon as add(gc) lands (auto RAW deps)
            pass_of = {dc: gp for gp, dts in enumerate(GDTS) for dc in dts}
            for dc in range(DC):
                gst = mpool.tile([P, 1, D], F32R, tag="gst", bufs=2)
                rd = nc.scalar.dma_start(
                    gst[:], Gsh[bass.ds(svp, 1), dc * P:(dc + 1) * P, :]
                    .rearrange("s (c p) n -> p (s c) n", p=P))
                add_dep_helper(rd.ins, barriers[pass_of[dc]].ins,
                               reason="peer G after pair barrier")
                nc.vector.tensor_add(gsb[:, dc], gsb[:, dc], gst[:, 0])

        # ------------- Phase V1 = G @ WvT, then A = PT^T @ V1 --------------
        # contraction-outer waves of 8 PSUM banks: each wave consumes G/V1/A
        # chunks in arrival order, so the whole chain soft-pipelines behind
        # the pair exchange instead of waiting for the full predecessor
        with ExitStack() as actx:
            vpool = actx.enter_context(tc.tile_pool(name="vpool", bufs=1))
            apool = actx.enter_context(tc.tile_pool(name="apool", bufs=1))
            xtpool = actx.enter_context(tc.tile_pool(name="xtpool", bufs=4))
            opool = actx.enter_context(tc.tile_pool(name="opool", bufs=2))

            # V1 runs as 4 waves of 4 PSUM banks, two waves in flight, with
            # each pair of live waves consuming own-pass chunks (gc 0..5)
            # before the late-arriving tail chunks (gc 6..7): the PE stays
            # busy on issued work while the last peer exchange lands
            v1 = vpool.tile([P, DC, D], F32R, tag="v1")
            GEARLY = 6
            ps = {}
            for w in range(4):
                for dt in range(w * 2, w * 2 + 2):
                    for dh in range(2):
                        ps[dt, dh] = psum.tile([P, FREE], F32, tag="ps",
                                               name=f"v1_{dt}_{dh}")

            def v1_mm(dt, dh, gc):
                # lhsT wants G[g, d-tile]; G is symmetric so the row-chunk
                # gc doubles as the column chunk
                nc.tensor.matmul(
                    ps[dt, dh][:], gsb[:, gc, dt * P:(dt + 1) * P],
                    wv[:, gc, dh * FREE:(dh + 1) * FREE],
                    start=(gc == 0), stop=(gc == DC - 1))

            for wpairbase in (0, 4):  # waves {0,1} then {2,3} (dt pairs)
                for gcs in (range(0, GEARLY), range(GEARLY, DC)):
                    for dt in range(wpairbase, wpairbase + 4):
                        for gc in gcs:
                            for dh in range(2):
                                v1_mm(dt, dh, gc)
                for dt in range(wpairbase, wpairbase + 4):
                    for dh in range(2):
                        nc.vector.tensor_copy(
                            v1[:, dt, dh * FREE:(dh + 1) * FREE],
                            ps[dt, dh][:])

            asb = apool.tile([P, DC, D], F32R, tag="asb")
            for w in range(2):
                ets = range(w * 4, (w + 1) * 4)
                ps = {(et, dh): psum.tile([P, FREE], F32, tag="ps",
                                          name=f"a_{et}_{dh}")
                      for et in ets for dh in range(2)}
                for dc in range(DC):
                    for et in ets:
                        for dh in range(2):
                            nc.tensor.matmul(
                                ps[et, dh][:],
                                pt[:, dc, et * P:(et + 1) * P],
                                v1[:, dc, dh * FREE:(dh + 1) * FREE],
                                start=(dc == 0), stop=(dc == DC - 1))
                for et in ets:
                    for dh in range(2):
                        nc.vector.tensor_copy(
                            asb[:, et, dh * FREE:(dh + 1) * FREE],
                            ps[et, dh][:])

            # ------------- Phase out = x_half @ A, streamed per l-tile -----
            for w in range(LH // P // 4):
                lts = range(w * 4, (w + 1) * 4)
                xts = {}
                for lt in lts:
                    xts[lt] = xtpool.tile([P, DC, P], F32R, tag="xt",
                                          name=f"xt_{lt}")
                    nc.scalar.dma_start(
                        xts[lt][:],
                        chunked(xTh).bitcast(F32R)[:, :, lt * P:(lt + 1) * P])
                ps = {(lt, dh): psum.tile([P, FREE], F32, tag="ps",
                                          name=f"o_{lt}_{dh}")
                      for lt in lts for dh in range(2)}
                for ec in range(DC):
                    for lt in lts:
                        for dh in range(2):
                            nc.tensor.matmul(
                                ps[lt, dh][:], xts[lt][:, ec],
                                asb[:, ec, dh * FREE:(dh + 1) * FREE],
                                start=(ec == 0), stop=(ec == DC - 1))
                for lt in lts:
                    ot = opool.tile([P, D], F32, tag="ot")
                    for dh in range(2):
                        nc.vector.tensor_copy(
                            ot[:, dh * FREE:(dh + 1) * FREE], ps[lt, dh][:])
                    nc.sync.dma_start(out[lt * P:(lt + 1) * P, :], ot[:])

    nc.compile()
    return nc


_NC_CACHE = {}


def _get_nc():
    if "nc" not in _NC_CACHE:
        _NC_CACHE["nc"] = build_nc()
    return _NC_CACHE["nc"]


def run(inputs, trace=False):
    """Run the kernel on all 8 cores. Returns (full_output, BassKernelResults)."""
    x = np.asarray(inputs["x"], dtype=np.float32)
    Wq = np.asarray(inputs["Wq"], dtype=np.float32)
    Wk = np.asarray(inputs["Wk"], dtype=np.float32)
    Wv = np.asarray(inputs["Wv"], dtype=np.float32)

    inv_sqrt_d = np.float32(1.0 / np.sqrt(D))
    ptm = np.ascontiguousarray(Wk.T @ (Wq * inv_sqrt_d))  # PT[d,e]
    wvT = np.ascontiguousarray(Wv.T)

    in_maps = []
    for c in range(N_CORES):
        b, h = c // 2, c % 2
        xh = np.ascontiguousarray(x[b, h * LH:(h + 1) * LH, :])
        in_maps.append({
            "xb": xh,
            "xTh": np.ascontiguousarray(xh.T),
            "slots": np.array([[h, 1 - h]], dtype=np.uint32),
            "wvT": wvT, "ptm": ptm,
        })

    nc = _get_nc()
    res = run_bass_kernel_spmd(nc, in_maps, list(range(N_CORES)), trace=trace)

    full = np.empty((B, L, D), dtype=np.float32)
    for c in range(N_CORES):
        b, h = c // 2, c % 2
        full[b, h * LH:(h + 1) * LH, :] = res.results[c]["out"]
    return full, res


def kernel(**inputs):
    full, _ = run(inputs, trace=False)
    return full


# revision 16
# speedup vs baseline: 1.2285x; 1.2285x over previous
"""Trainium2 Bass kernel for softmax-free attention:
    q = x @ Wq^T; k = x @ Wk^T; v = x @ Wv^T
    s = (q @ k^T) / sqrt(d); out = s @ v
  x: [4, 4096, 1024], W*: [1024, 1024], out: [4, 4096, 1024] (fp32)

No softmax => the whole map is linear and can be re-associated:
    out[b] = x[b] @ A[b],  A[b] = PT^T @ (G[b] @ Wv^T),
    G[b] = x[b]^T x[b],    PT = Wk^T (Wq/sqrt(d))   (host-folded weights)
This cuts device MACs ~4x vs the naive chain (projections + LxL scores).

Sharding: 8 cores; core c handles batch b=c//2, sequence-half h=c%2.  Each
pair member computes the partial gram G_part = x_half^T x_half over its OWN
2048 rows in two PSUM passes split by COLUMN half, spilling each finished
half to a cross-core-visible Shared-DRAM slot; the peer's partial is read
back and summed chunk-wise.  Pass 1's exchange is fully hidden under pass
2's compute: V1 groups dt0-3 only need G columns 0:512.  Ordering across
the pair is a tiny token AllReduce per spill (the token is DMA-sampled from
the shared buffer, so it carries a RAW dep on the spill); only peer reads
wait on it.

The post-G chain is split across the pair by output-column half (column
splits survive left-multiplication):
    V1h = G @ WvT[:, own 512 cols]      (64 matmuls instead of 128)
    A_own = PT^T @ V1h                  (64 matmuls, gives A[:, own cols])
A halves are exchanged through Shared DRAM under a third token barrier,
hidden beneath out's own-half waves: out = x_half @ [A_own | A_peer] with
columns in own-first rotated order, un-rotated on the host.

Layout: the PE contracts over the partition dim; every matmul is arranged so
its output lands partition-major for the next stage:
  G[d,g]   = sum_l  x[l,d] x[l,g]      (lhsT=x chunk col-slice, rhs=x chunk)
  V1[d,d'] = sum_g  G[g,d] WvT[g,d']   (lhsT=G row-chunk == col-chunk, sym!)
  A[e,d']  = sum_d  PT[d,e] V1[d,d']   (lhsT=PT chunk, rhs=V1 chunk)
  out[l,d']= sum_e  xT[e,l] A[e,d']    (lhsT=xT_half chunk, rhs=A chunk)
All matmul inputs are float32r (full PE rate at free-dim>=256, ~1e-4 rel
err); PSUM accumulates fp32.  DMA ring assignment keeps every hardware
queue FIFO-consistent: barrier-gated reads are always queued after the
ungated traffic they would otherwise block.
"""

import sys
import types
from contextlib import ExitStack

import numpy as np

import concourse.bass as bass
import concourse.tile as tile
from concourse import bacc, mybir
from concourse.bass_utils import run_bass_kernel_spmd
from concourse.mybir import EngineType
from concourse.tile import add_dep_helper
from concourse.vector_clock import ScopedClock

# ---------------------------------------------------------------------------
# Environment shims
# ---------------------------------------------------------------------------


def _install_tile_drain_patch():
    """This toolchain's walrus caps sync waits at 1 per instruction, but
    TileContext's tail drain can carry several. Split the overflow onto
    preceding nops (same semantics: the issuing engine observes every sem
    before draining)."""
    if getattr(tile.TileContext, "_drain_patch_installed", False):
        return

    def _patched_drain_and_barrier(self, tick_clock, wait_clock):
        nc = self.nc
        collector = nc.sync.nop(hint="drain_wait_collector", nofuse=True)
        wait_clock.add_sem_waits(
            collector.ins, ScopedClock({None: tick_clock.global_clock})
        )
        waits = list(collector.ins.sync_info.on_wait or [])
        if len(waits) > 1:
            collector.ins.sync_info.on_wait = [waits[0]]
            for w in waits[1:]:
                nop = nc.sync.nop(hint="drain_wait_extra", nofuse=True)
                nop.ins.sync_info = mybir.SyncInfo(on_wait=[w], on_update=[])
        nc.sync.drain()

        nc.all_engine_barrier()
        assert self.sems is not None
        popped = nc._tile_sem_poison_stack.pop()
        assert popped is self._sem_poison
        nc.clear_and_free_semaphores(list(self.sems.allocated().values()))
        nc.all_engine_barrier()

    tile.TileContext._drain_and_barrier = _patched_drain_and_barrier
    tile.TileContext._drain_patch_installed = True


def _install_ntff_shim():
    """The image's antenv lacks axon_hooks, which silently degrades
    trace=True. Recreate the get/set pair and register the ctypes NTFF hook
    from trn_agent_boot (no-op if unavailable)."""
    if "antenv.axon_hooks" in sys.modules:
        return
    state = {"hook": None}

    def set_axon_ntff_profile_hook(h):
        state["hook"] = h

    def get_axon_ntff_profile_hook():
        return state["hook"]

    mod = types.ModuleType("antenv.axon_hooks")
    mod.set_axon_ntff_profile_hook = set_axon_ntff_profile_hook
    mod.get_axon_ntff_profile_hook = get_axon_ntff_profile_hook
    sys.modules["antenv.axon_hooks"] = mod
    try:
        import antenv

        antenv.axon_hooks = mod
        from trn_agent_boot.trn_boot import _ntff_profile_via_ctypes

        set_axon_ntff_profile_hook(
            _ntff_profile_via_ctypes("/opt/axon/libaxon_pjrt.so")
        )
    except Exception:
        pass


_install_tile_drain_patch()
_install_ntff_shim()

# ---------------------------------------------------------------------------
# Problem constants (hardcoded per the harness contract)
# ---------------------------------------------------------------------------

B, L, D = 4, 4096, 1024
N_CORES = 8
P = 128
LH = L // 2  # rows per core
DC = D // P  # 8 chunks of 128 over d/e/g
F32 = mybir.dt.float32
F32R = mybir.dt.float32r
FREE = 512  # rhs free dim per matmul (one PSUM bank)

PAIRS = [[2 * i, 2 * i + 1] for i in range(N_CORES // 2)]
LCH = LH // P  # 16 l-chunks of own half
NBAR = 3       # pair barriers: G col-half 0, G col-half 1, A halves


def build_nc():
    nc = bacc.Bacc("TRN2", target_bir_lowering=False, debug=False,
                   num_devices=N_CORES)
    xb = nc.dram_tensor("xb", [LH, D], F32, kind="ExternalInput").ap()
    xTh = nc.dram_tensor("xTh", [D, LH], F32, kind="ExternalInput").ap()
    wvTh = nc.dram_tensor("wvTh", [D, FREE], F32, kind="ExternalInput").ap()
    ptm = nc.dram_tensor("ptm", [D, D], F32, kind="ExternalInput").ap()
    out = nc.dram_tensor("out", [LH, D], F32, kind="ExternalOutput").ap()
    slots = nc.dram_tensor("slots", [1, 2], mybir.dt.uint32,
                           kind="ExternalInput").ap()
    Gsh = nc.dram_tensor("Gsh", [2, D, D], F32R, addr_space="Shared").ap()
    Ash = nc.dram_tensor("Ash", [2, D, FREE], F32R, addr_space="Shared").ap()
    toks = [nc.dram_tensor(f"tok{p}", [1, 1], F32).ap() for p in range(NBAR)]
    toks2 = [nc.dram_tensor(f"tok2{p}", [1, 1], F32).ap()
             for p in range(NBAR)]
    wu_sink = nc.dram_tensor("wu_sink", [P, FREE], F32).ap()

    def chunked(ap):  # [K*, N] dram -> [P, K*/P, N] partition-major
        return ap.rearrange("(c p) n -> p c n", p=P)

    with tile.TileContext(nc) as tc, ExitStack() as octx:
        psum = octx.enter_context(tc.tile_pool(name="psum", bufs=8,
                                               space="PSUM"))
        wpool = octx.enter_context(tc.tile_pool(name="wpool", bufs=1))
        apool = octx.enter_context(tc.tile_pool(name="apool", bufs=1))
        mpool = octx.enter_context(tc.tile_pool(name="mpool", bufs=2))

        wv = wpool.tile([P, DC, FREE], F32R, tag="wv")  # WvT own-col half
        pt = wpool.tile([P, DC, D], F32R, tag="pt")
        asb = apool.tile([P, DC, D], F32R, tag="asb")  # [A_own | A_peer]
        nc.gpsimd.dma_start(wv[:], chunked(wvTh).bitcast(F32R))
        nc.gpsimd.dma_start(pt[:], chunked(ptm).bitcast(F32R))

        # rank-in-pair slot selectors for the shared spill buffers
        st_sl = mpool.tile([1, 2], mybir.dt.uint32, tag="sl", bufs=1)
        nc.sync.dma_start(st_sl[:], slots[:])
        regs_o = nc.alloc_registers(
            engines=[EngineType.SP, EngineType.Activation])
        nc.regs_load(regs_o, st_sl[0:1, 0:1])
        svo = nc.snap(regs_o, donate=True)
        regs_p = nc.alloc_registers(
            engines=[EngineType.SP, EngineType.Activation])
        nc.regs_load(regs_p, st_sl[0:1, 1:2])
        svp = nc.snap(regs_p, donate=True)

        barriers = [None] * NBAR

        def pair_barrier(bi, shared_probe):
            # token sampled from the shared buffer carries a RAW dep on the
            # spill; the AllReduce completes only when BOTH pair members'
            # spills are durable
            tkt = mpool.tile([1, 1], F32, tag=f"tkt{bi}", bufs=1,
                             name=f"tkt{bi}")
            nc.sync.dma_start(tkt[0:1, 0:1], shared_probe)
            nc.sync.dma_start(toks[bi][:], tkt[0:1, 0:1])
            barriers[bi] = nc.gpsimd.collective_compute(
                "AllReduce", mybir.AluOpType.add, replica_groups=PAIRS,
                ins=[toks[bi]], outs=[toks2[bi]])

        with ExitStack() as gctx:
            gpool = gctx.enter_context(tc.tile_pool(name="gpool", bufs=1))
            vpool = gctx.enter_context(tc.tile_pool(name="vpool", bufs=1))
            gsb = gpool.tile([P, DC, D], F32R, tag="gsb")  # full G after add
            v1h = vpool.tile([P, DC, FREE], F32R, tag="v1h")

            with ExitStack() as xctx:
                wupool = xctx.enter_context(tc.tile_pool(name="wupool",
                                                         bufs=1))
                xpool = xctx.enter_context(tc.tile_pool(name="xpool",
                                                        bufs=8))

                # HAM warmup: junk matmuls while the first DMAs load, so the
                # PE clock gate is already at 8/8 when real work arrives
                wut = wupool.tile([P, FREE], F32R, tag="wut")
                nc.vector.memset(wut[:].bitcast(F32), 0.0)
                wuo = wupool.tile([P, FREE], F32, tag="wuo")
                for g in range(20):
                    wp = psum.tile([P, FREE], F32, tag="ps", name=f"wu_{g}")
                    for r in range(2):
                        nc.tensor.matmul(wp[:], wut[:, 0:P], wut[:],
                                         start=(r == 0), stop=(r == 1))
                    if g == 19:
                        nc.vector.tensor_copy(wuo[:], wp[:])
                nc.sync.dma_start(wu_sink[:], wuo[:])

                # ---- Phase G: partial gram, two passes by COLUMN half ----
                # pass gh computes G[:, gh*512:(gh+1)*512] for all 1024 rows
                # (8 PSUM banks x 16 l-chunks); x is re-streamed per pass
                for gh in range(2):
                    gcols = slice(gh * FREE, (gh + 1) * FREE)
                    gps = {}
                    for dt in range(DC):
                        gps[dt] = psum.tile([P, FREE], F32, tag="ps",
                                            name=f"g_{gh}_{dt}")
                    for lc in range(LCH):
                        xc = xpool.tile([P, 1, D], F32R, tag="xc",
                                        name=f"xc_{gh}_{lc}")
                        nc.scalar.dma_start(
                            xc[:], chunked(xb).bitcast(F32R)[:, lc:lc + 1])
                        for dt in range(DC):
                            nc.tensor.matmul(
                                gps[dt][:],
                                xc[:, 0, dt * P:(dt + 1) * P],
                                xc[:, 0, gcols],
                                start=(lc == 0), stop=(lc == LCH - 1))
                    for dt in range(DC):
                        nc.vector.tensor_copy(gsb[:, dt, gcols], gps[dt][:])
                    # bulk spill of this column half, then its pair barrier
                    nc.sync.dma_start(
                        Gsh[bass.ds(svo, 1), :, gcols].rearrange(
                            "s (c p) n -> p (s c) n", p=P),
                        gsb[:, :, gcols])
                    pair_barrier(gh, Gsh[
                        bass.ds(svo, 1), 0:1, gh * FREE:gh * FREE + 1]
                        .rearrange("s c n -> c s n").bitcast(F32))

                # read peer partial per (half, chunk) and add into gsb; V1h
                # below consumes each column half as its adds land
                for gh in range(2):
                    gcols = slice(gh * FREE, (gh + 1) * FREE)
                    for gc in range(DC):
                        gst = mpool.tile([P, 1, FREE], F32R, tag="gst",
                                         bufs=2, name=f"gst_{gh}_{gc}")
                        rd = nc.sync.dma_start(
                            gst[:],
                            Gsh[bass.ds(svp, 1), gc * P:(gc + 1) * P, gcols]
                            .rearrange("s (c p) n -> p (s c) n", p=P))
                        add_dep_helper(rd.ins, barriers[gh].ins,
                                       reason="peer G after pair barrier")
                        nc.vector.tensor_add(gsb[:, gc, gcols],
                                             gsb[:, gc, gcols], gst[:, 0])

            # ---- Phase V1h = G @ WvT[:, own half] ------------------------
            # group dt's lhsT slices G columns dt*128..: dt 0-3 only need
            # column half 0 (enabled by barrier 0 while pass 1 computes)
            psv = {}
            for dt in range(DC):
                psv[dt] = psum.tile([P, FREE], F32, tag="ps",
                                    name=f"v1_{dt}")
            for dt in range(DC):
                for gc in range(DC):
                    # lhsT wants G[g, d-tile]; G is symmetric so the row
                    # chunk gc doubles as the column chunk
                    nc.tensor.matmul(
                        psv[dt][:], gsb[:, gc, dt * P:(dt + 1) * P],
                        wv[:, gc, :],
                        start=(gc == 0), stop=(gc == DC - 1))
                nc.vector.tensor_copy(v1h[:, dt, :], psv[dt][:])

            # ---- Phase A_own = PT^T @ V1h  (A columns = own half) --------
            psa = {}
            for et in range(DC):
                psa[et] = psum.tile([P, FREE], F32, tag="ps",
                                    name=f"a_{et}")
            for dc in range(DC):
                for et in range(DC):
                    nc.tensor.matmul(
                        psa[et][:], pt[:, dc, et * P:(et + 1) * P],
                        v1h[:, dc, :],
                        start=(dc == 0), stop=(dc == DC - 1))
            for et in range(DC):
                nc.vector.tensor_copy(asb[:, et, 0:FREE], psa[et][:])

            # spill A_own and raise the third pair barrier
            nc.sync.dma_start(
                Ash[bass.ds(svo, 1), :, :].rearrange(
                    "s (c p) n -> p (s c) n", p=P),
                asb[:, :, 0:FREE])
            pair_barrier(2, Ash[bass.ds(svo, 1), 0:1, 0:1]
                         .rearrange("s c n -> c s n").bitcast(F32))

        # ---- Phase out = x_half @ [A_own | A_peer] -----------------------
        # own-column waves run first (they only need A_own), covering the A
        # exchange; peer-column waves consume A_peer chunks as reads land
        with ExitStack() as octx2:
            xtpool = octx2.enter_context(tc.tile_pool(name="xtpool",
                                                      bufs=16))
            opool = octx2.enter_context(tc.tile_pool(name="opool", bufs=4))

            xts = {}
            for lt in range(LH // P):
                xts[lt] = xtpool.tile([P, DC, P], F32R, tag="xt",
                                      name=f"xt_{lt}")
                nc.scalar.dma_start(
                    xts[lt][:],
                    chunked(xTh).bitcast(F32R)[:, :, lt * P:(lt + 1) * P])

            # peer A chunks: issued after the xt loads on the same ring so
            # the barrier gate never blocks ungated traffic
            for ec in range(DC):
                rd = nc.scalar.dma_start(
                    asb[:, ec:ec + 1, FREE:D],
                    Ash[bass.ds(svp, 1), ec * P:(ec + 1) * P, :]
                    .rearrange("s (c p) n -> p (s c) n", p=P))
                add_dep_helper(rd.ins, barriers[2].ins,
                               reason="peer A after pair barrier")

            for lts, dh in ((range(0, 8), 0), (range(8, 16), 0),
                            (range(0, 8), 1), (range(8, 16), 1)):
                acols = slice(dh * FREE, (dh + 1) * FREE)
                pso = {}
                for lt in lts:
                    pso[lt] = psum.tile([P, FREE], F32, tag="ps",
                                        name=f"o_{lt}_{dh}")
                for ec in range(DC):
                    for lt in lts:
                        nc.tensor.matmul(
                            pso[lt][:], xts[lt][:, ec], asb[:, ec, acols],
                            start=(ec == 0), stop=(ec == DC - 1))
                for lt in lts:
                    ot = opool.tile([P, FREE], F32, tag="ot",
                                    name=f"ot_{lt}_{dh}")
                    nc.vector.tensor_copy(ot[:], pso[lt][:])
                    nc.sync.dma_start(
                        out[lt * P:(lt + 1) * P, acols], ot[:])

    nc.compile()
    return nc


_NC_CACHE = {}


def _get_nc():
    if "nc" not in _NC_CACHE:
        _NC_CACHE["nc"] = build_nc()
    return _NC_CACHE["nc"]


def run(inputs, trace=False):
    """Run the kernel on all 8 cores. Returns (full_output, BassKernelResults)."""
    x = np.asarray(inputs["x"], dtype=np.float32)
    Wq = np.asarray(inputs["Wq"], dtype=np.float32)
    Wk = np.asarray(inputs["Wk"], dtype=np.float32)
    Wv = np.asarray(inputs["Wv"], dtype=np.float32)

    inv_sqrt_d = np.float32(1.0 / np.sqrt(D))
    ptm = np.ascontiguousarray(Wk.T @ (Wq * inv_sqrt_d))  # PT[d,e]
    wvT = np.ascontiguousarray(Wv.T)

    in_maps = []
    for c in range(N_CORES):
        b, h = c // 2, c % 2
        xh = np.ascontiguousarray(x[b, h * LH:(h + 1) * LH, :])
        in_maps.append({
            "xb": xh,
            "xTh": np.ascontiguousarray(xh.T),
            "slots": np.array([[h, 1 - h]], dtype=np.uint32),
            "wvTh": np.ascontiguousarray(wvT[:, h * FREE:(h + 1) * FREE]),
            "ptm": ptm,
        })

    nc = _get_nc()
    res = run_bass_kernel_spmd(nc, in_maps, list(range(N_CORES)), trace=trace)

    full = np.empty((B, L, D), dtype=np.float32)
    for c in range(N_CORES):
        b, h = c // 2, c % 2
        rows = slice(h * LH, (h + 1) * LH)
        o = res.results[c]["out"]
        # un-rotate the own-first column order
        full[b, rows, h * FREE:(h + 1) * FREE] = o[:, 0:FREE]
        full[b, rows, (1 - h) * FREE:(2 - h) * FREE] = o[:, FREE:D]
    return full, res


def kernel(**inputs):
    full, _ = run(inputs, trace=False)
    return full
